# revision 25
# baseline (speedup 1.0000x reference)
"""Multi-level DWT (DB4) decomposition on 8 Trainium2 NeuronCores — v2.

Strategy
--------
Data-parallel across cores (512 batch rows/core), TRANSPOSED on-core layout:
the wavelet axis lives on SBUF partitions and the 512 batch rows are every
matmul's free dim.  The 11 levels collapse into two passes of banded matrix
products, built numerically on the host from the provided W:

  Pass 1 (levels 0-4): out = T_ext^T @ xT, where T_ext [4098, 4096] is the
  5-level composition (wrap rows 4096/4097 hold the level-0 wraparound taps,
  deeper-level zero-truncation baked in).  Columns are packed so out-tile J
  = [A5(4) | D5(4) | D4(8) | D3(16) | D2(32) | D1(64)] covers input rows
  [128J, 128J+190) only: one main matmul (tile J) + one accumulating edge
  matmul (tile J+1) per out-tile.  Interior tiles share one (S_main, S_edge)
  stationary pair; tile 31 gets its own (truncation + wrap).
  Pass 2 (levels 5-10): one dense 128x128 stationary G maps a5 -> y[0:128].

  65 matmuls total (~14 us PE) vs ~130k PE cycles for the per-tap scaled-
  identity formulation.  fp16 end-to-end (error ~3.5e-4 << 2e-2 gate) halves
  HBM traffic to ~8.4 MB/core: the kernel is DMA-bound at ~24 us.

  Drains are 33 full-width [128,512] PSUM->SBUF casts split over DVE/ACT.
  Detail coefficients leave straight from the staged tiles via strided
  gather-DMAs (partition range of every staged tile -> contiguous DRAM
  rows), chunked so output streaming overlaps pass-1 compute.  Host does
  the transposes / fp16 casts (excluded from HW time).
"""
import sys

if "/opt/trn_rl_repo" not in sys.path:
    sys.path.insert(0, "/opt/trn_rl_repo")

import numpy as np

import concourse.bacc as bacc
import concourse.mybir as mybir
from concourse import tile
from concourse.bass_utils import run_bass_kernel_spmd

DB4 = [0.4829629131445341, 0.8365163037378079, 0.2241438680420134,
       -0.1294095225512604]

B, N = 4096, 4096
NCORES = 8
RPC = B // NCORES        # rows per core = 512 (matmul free dim)
P = 128
NT = N // P              # pass-1 tiles = 32
XR = N + 2               # xT rows incl wrap = 4098

F16 = mybir.dt.float16
F32 = mybir.dt.float32

_nc_cache = {}
_stat_cache = {}


def _build_matrix(c, n):
    m = np.zeros((n, n), dtype=np.float64)
    m[-2:, 0:2] = np.array([[c[2], c[3]], [c[1], -c[0]]])
    m[-2:, -2:] = np.array([[c[0], c[1]], [c[3], -c[2]]])
    shift = 0
    for i in range(0, n - 2, 2):
        m[i, shift:shift + 4] = np.array(c)
        m[i + 1, shift:shift + 4] = np.array([c[3], -c[2], c[1], -c[0]])
        shift += 2
    return m.T


def _col_index(J, s):
    """Global pass-1 out column for slot s of out-tile J."""
    if s < 4:
        return 4 * J + s                    # A5
    if s < 8:
        return 128 + 4 * J + (s - 4)        # D5
    if s < 16:
        return 256 + 8 * J + (s - 8)        # D4
    if s < 32:
        return 512 + 16 * J + (s - 16)      # D3
    if s < 64:
        return 1024 + 32 * J + (s - 32)     # D2
    return 2048 + 64 * J + (s - 64)         # D1


def _stationaries(W=None):
    """Build the 5 stationaries [128, 640] fp16: Sm | Se | Sm31 | Se31 | G."""
    key = None if W is None else hash(np.asarray(W)[:4, :4].tobytes())
    if key in _stat_cache:
        return _stat_cache[key]
    if W is None:
        Wf = _build_matrix(DB4, N).astype(np.float32)
    else:
        Wf = np.asarray(W, np.float32)

    # T_ext: [4098, 4096], wrap taps moved to rows 4096/4097
    W0e = np.zeros((XR, N), np.float32)
    W0e[:N] = Wf
    for j in (N - 2, N - 1):
        for i in (0, 1):
            W0e[N + i, j] = Wf[i, j]
            W0e[i, j] = 0.0
    T = np.concatenate([W0e[:, 0::2], W0e[:, 1::2]], axis=1)
    for lev in range(1, 5):
        L = N >> lev
        y = T[:, :L] @ Wf[:L, :L]
        T[:, :L] = np.concatenate([y[:, 0::2], y[:, 1::2]], axis=1)

    U = np.eye(128, dtype=np.float32)
    for lev in range(5, 11):
        L = N >> lev
        y = U[:, :L] @ Wf[:L, :L]
        U[:, :L] = np.concatenate([y[:, 0::2], y[:, 1::2]], axis=1)

    cols0 = np.array([_col_index(0, s) for s in range(128)])
    cols31 = np.array([_col_index(31, s) for s in range(128)])
    Sm = T[0:128, cols0]
    Se = T[128:256, cols0]
    Sm31 = T[128 * 31:128 * 31 + 128, cols31]
    Se31 = np.zeros((128, 128), np.float32)
    Se31[0:2] = T[N:XR, cols31]

    # odd tiles use the packing rolled by 64 partitions so segment
    # gather-DMAs can pair port-disjoint even/odd partition ranges
    def roll(S):
        return np.roll(S, 64, axis=1)

    w = np.concatenate([Sm, Se, roll(Sm), roll(Se), roll(Sm31), roll(Se31),
                        U], axis=1).astype(np.float16)
    _stat_cache[key] = w
    return w


def build_program(loop_iters=None, variant="full"):
    """Build + compile the per-core Bass program (identical on all cores)."""
    key = (loop_iters, variant)
    if key in _nc_cache:
        return _nc_cache[key]
    mm_only = variant == "mm"

    nc = bacc.Bacc("TRN2", target_bir_lowering=False, debug=False)
    x_d = nc.dram_tensor("x", [XR, RPC], F16, kind="ExternalInput").ap()
    w_d = nc.dram_tensor("w", [P, 7 * P], F16, kind="ExternalInput").ap()
    y_d = nc.dram_tensor("y", [N, RPC], F16, kind="ExternalOutput").ap()
    a5s_d = nc.dram_tensor("a5s", [P, RPC], F16, kind="Internal").ap()

    with tile.TileContext(nc) as tc:
        with tc.tile_pool(name="sb", bufs=1) as sb, \
             tc.tile_pool(name="ps", bufs=8, space="PSUM") as ps:
            x_t = sb.tile([P, 33 * RPC], F16, name="x_t")
            st_t = sb.tile([P, NT * RPC], F16, name="st_t")
            w_t = sb.tile([P, 7 * P], F16, name="w_t")
            a5_t = sb.tile([P, RPC], F16, name="a5_t")
            p2_t = sb.tile([P, RPC], F16, name="p2_t")

            def xt(J):
                return x_t[:, J * RPC:(J + 1) * RPC]

            def stt(J):
                return st_t[:, J * RPC:(J + 1) * RPC]

            def body(_iv=None):
                nc.sync.dma_start(w_t[:], w_d)
                # batched input DMAs (HWDGE fixed cost is ~630 ns per
                # dma_start — per-tile DMAs serialize on it); small first
                # batch so the pipeline starts early
                batches = [(0, 2), (2, 6), (8, 8), (16, 8), (24, 8)]
                for j0, nj in batches:
                    sv = x_d[j0 * P:(j0 + nj) * P, :].rearrange(
                        "(j p) f -> p j f", p=P)
                    dv = x_t[:, j0 * RPC:(j0 + nj) * RPC].rearrange(
                        "p (j f) -> p j f", j=nj)
                    nc.sync.dma_start(dv, sv)
                # wrap tile 32: rows 0,1 = x cols 0,1; rest zero (only the
                # J=31 edge matmul reads it — keep off the startup path)
                nc.vector.memset(x_t[:, 32 * RPC:33 * RPC], 0.0)
                nc.sync.dma_start(x_t[0:2, 32 * RPC:33 * RPC], x_d[N:XR, :])

                # PE clock warmup (HAM un-throttles after ~3.4 us of busy;
                # any idle gap re-throttles, so bridge until batch 0 lands)
                pw = ps.tile([P, RPC], F32, name="pch", tag="ps")
                for _ in range(12):
                    nc.tensor.matmul(pw[:], w_t[:, 0:P], w_t[:, 0:512],
                                     start=True, stop=True)

                def a5_scatter(q):
                    """a5s row 4J+p <- staged tile J partition 64*(J%2)+p,
                    for tiles of parity q.  Tile auto-sems the DRAM RAW dep.
                    Sync ring: the scalar ring is congested with the late
                    gather chunks, which would delay the pass-2 readback."""
                    sv = st_t[64 * q:64 * q + 4, :].rearrange(
                        "p (j o f) -> p j o f", j=NT // 2, o=2)[:, :, q:q + 1, :]
                    dv = a5s_d.rearrange("(j o p) f -> p j o f",
                                         o=2, p=4)[:, :, q:q + 1, :]
                    nc.sync.dma_start(dv, sv)

                def seg_gather(pbase, seglen, j0, ntiles, dbase, eng=None):
                    """Drain one segment for tiles [j0, j0+ntiles) as an
                    even-tile + odd-tile DMA pair on disjoint SBUF ports."""
                    j2 = ntiles // 2
                    for par in (0, 1):
                        pb = (pbase + 64 * par) % P
                        sv = st_t[pb:pb + seglen,
                                  j0 * RPC:(j0 + ntiles) * RPC].rearrange(
                            "p (j q f) -> p j q f", j=j2, q=2)[:, :, par:par + 1, :]
                        dv = y_d[dbase + seglen * j0:
                                 dbase + seglen * (j0 + ntiles), :].rearrange(
                            "(j q p) f -> p j q f", q=2, p=seglen)[:, :, par:par + 1, :]
                        (eng or nc.scalar).dma_start(dv, sv)

                # pass 1
                for J in range(NT):
                    par = J % 2
                    mo = 2 * par if J < NT - 1 else 4
                    pt = ps.tile([P, RPC], F32, name="pch", tag="ps")
                    nc.tensor.matmul(pt[:], w_t[:, mo * P:(mo + 1) * P],
                                     xt(J), start=True, stop=False)
                    nc.tensor.matmul(pt[:], w_t[:, (mo + 1) * P:(mo + 2) * P],
                                     xt(J + 1), start=False, stop=True)
                    if mm_only:
                        continue
                    if J % 3 < 2:
                        nc.vector.tensor_copy(stt(J), pt[:])
                    else:
                        nc.scalar.copy(stt(J), pt[:])

                    # stream detail coeffs out as their staged tiles complete
                    if J in (7, 15, 23, 31):
                        seg_gather(64, 64, J - 7, 8, 2048)       # D1 chunk
                    if J in (15, 31):
                        j0 = J - 15
                        seg_gather(32, 32, j0, 16, 1024)         # D2 chunk
                        # small segments ride SWDGE (Pool) — off the
                        # serialized HWDGE path (~630 ns per dma_start)
                        seg_gather(16, 16, j0, 16, 512, nc.gpsimd)   # D3
                        seg_gather(8, 8, j0, 16, 256, nc.gpsimd)     # D4
                        seg_gather(4, 4, j0, 16, 128, nc.gpsimd)     # D5
                    if J == 30 and not mm_only:
                        a5_scatter(0)

                # (a5 odd-tile scatter + readback; even scatter went at J=30)
                if mm_only:
                    return
                a5_scatter(1)
                nc.sync.dma_start(a5_t[:], a5s_d)

                # pass 2: levels 5-10 in one dense matmul
                p2 = ps.tile([P, RPC], F32, name="pch", tag="ps")
                nc.tensor.matmul(p2[:], w_t[:, 6 * P:7 * P], a5_t[:],
                                 start=True, stop=True)
                nc.vector.tensor_copy(p2_t[:], p2[:])
                nc.scalar.dma_start(y_d[0:P, :], p2_t[:])

            if loop_iters is None:
                body()
            else:
                with tc.For_i(0, loop_iters, 1,
                              hint_engines=(mybir.EngineType.PE,)) as iv:
                    body(iv)

    nc.compile()
    _nc_cache[key] = nc
    return nc


def make_in_maps(x, W=None):
    """Host prep: per-core transposed fp16 inputs + stationaries."""
    x = np.asarray(x, np.float32)
    w_np = _stationaries(W)
    in_maps = []
    for c in range(NCORES):
        xc = x[c * RPC:(c + 1) * RPC]
        xT = np.empty((XR, RPC), np.float16)
        xT[:N] = np.ascontiguousarray(xc.T, dtype=np.float16)
        xT[N] = xc[:, 0].astype(np.float16)
        xT[N + 1] = xc[:, 1].astype(np.float16)
        in_maps.append({"x": xT, "w": w_np})
    return in_maps


def kernel(input, W=None, **_unused):
    x = np.asarray(input, np.float32)
    assert x.shape == (B, N), x.shape
    in_maps = make_in_maps(x, W)
    nc = build_program()
    res = run_bass_kernel_spmd(nc, in_maps, core_ids=list(range(NCORES)))
    out = np.empty((B, N), np.float32)
    for c in range(NCORES):
        out[c * RPC:(c + 1) * RPC] = res.results[c]["y"].T.astype(np.float32)
    return out


# revision 28
# speedup vs baseline: 1.2164x; 1.2164x over previous
"""Multi-level DWT (DB4) decomposition on 8 Trainium2 NeuronCores — v2.

Strategy
--------
Data-parallel across cores (512 batch rows/core), TRANSPOSED on-core layout:
the wavelet axis lives on SBUF partitions and the 512 batch rows are every
matmul's free dim.  The 11 levels collapse into two passes of banded matrix
products, built numerically on the host from the provided W:

  Pass 1 (levels 0-4): out = T_ext^T @ xT, where T_ext [4098, 4096] is the
  5-level composition (wrap rows 4096/4097 hold the level-0 wraparound taps,
  deeper-level zero-truncation baked in).  Columns are packed so out-tile J
  = [A5(4) | D5(4) | D4(8) | D3(16) | D2(32) | D1(64)] covers input rows
  [128J, 128J+190) only: one main matmul (tile J) + one accumulating edge
  matmul (tile J+1) per out-tile.  Interior tiles share one (S_main, S_edge)
  stationary pair; tile 31 gets its own (truncation + wrap).
  Pass 2 (levels 5-10): one dense 128x128 stationary G maps a5 -> y[0:128].

  65 matmuls total (~14 us PE) vs ~130k PE cycles for the per-tap scaled-
  identity formulation.  fp16 end-to-end (error ~3.5e-4 << 2e-2 gate) halves
  HBM traffic to ~8.4 MB/core: the kernel is DMA-bound at ~24 us.

  Drains are 33 full-width [128,512] PSUM->SBUF casts split over DVE/ACT.
  Detail coefficients leave straight from the staged tiles via strided
  gather-DMAs (partition range of every staged tile -> contiguous DRAM
  rows), chunked so output streaming overlaps pass-1 compute.  Host does
  the transposes / fp16 casts (excluded from HW time).
"""
import sys

if "/opt/trn_rl_repo" not in sys.path:
    sys.path.insert(0, "/opt/trn_rl_repo")

import numpy as np

import concourse.bacc as bacc
import concourse.mybir as mybir
from concourse import tile
from concourse.bass_utils import run_bass_kernel_spmd

DB4 = [0.4829629131445341, 0.8365163037378079, 0.2241438680420134,
       -0.1294095225512604]

B, N = 4096, 4096
NCORES = 8
RPC = B // NCORES        # rows per core = 512 (matmul free dim)
P = 128
NT = N // P              # pass-1 tiles = 32
XR = N + 2               # xT rows incl wrap = 4098

F16 = mybir.dt.float16
F32 = mybir.dt.float32

_nc_cache = {}
_stat_cache = {}


def _build_matrix(c, n):
    m = np.zeros((n, n), dtype=np.float64)
    m[-2:, 0:2] = np.array([[c[2], c[3]], [c[1], -c[0]]])
    m[-2:, -2:] = np.array([[c[0], c[1]], [c[3], -c[2]]])
    shift = 0
    for i in range(0, n - 2, 2):
        m[i, shift:shift + 4] = np.array(c)
        m[i + 1, shift:shift + 4] = np.array([c[3], -c[2], c[1], -c[0]])
        shift += 2
    return m.T


def _col_index(J, s):
    """Global pass-1 out column for slot s of out-tile J."""
    if s < 4:
        return 4 * J + s                    # A5
    if s < 8:
        return 128 + 4 * J + (s - 4)        # D5
    if s < 16:
        return 256 + 8 * J + (s - 8)        # D4
    if s < 32:
        return 512 + 16 * J + (s - 16)      # D3
    if s < 64:
        return 1024 + 32 * J + (s - 32)     # D2
    return 2048 + 64 * J + (s - 64)         # D1


def _stationaries(W=None):
    """Build the 5 stationaries [128, 640] fp16: Sm | Se | Sm31 | Se31 | G."""
    key = None if W is None else hash(np.asarray(W)[:4, :4].tobytes())
    if key in _stat_cache:
        return _stat_cache[key]
    if W is None:
        Wf = _build_matrix(DB4, N).astype(np.float32)
    else:
        Wf = np.asarray(W, np.float32)

    # T_ext: [4098, 4096], wrap taps moved to rows 4096/4097
    W0e = np.zeros((XR, N), np.float32)
    W0e[:N] = Wf
    for j in (N - 2, N - 1):
        for i in (0, 1):
            W0e[N + i, j] = Wf[i, j]
            W0e[i, j] = 0.0
    T = np.concatenate([W0e[:, 0::2], W0e[:, 1::2]], axis=1)
    for lev in range(1, 5):
        L = N >> lev
        y = T[:, :L] @ Wf[:L, :L]
        T[:, :L] = np.concatenate([y[:, 0::2], y[:, 1::2]], axis=1)

    U = np.eye(128, dtype=np.float32)
    for lev in range(5, 11):
        L = N >> lev
        y = U[:, :L] @ Wf[:L, :L]
        U[:, :L] = np.concatenate([y[:, 0::2], y[:, 1::2]], axis=1)

    cols0 = np.array([_col_index(0, s) for s in range(128)])
    cols31 = np.array([_col_index(31, s) for s in range(128)])
    Sm = T[0:128, cols0]
    Se = T[128:256, cols0]
    Sm31 = T[128 * 31:128 * 31 + 128, cols31]
    Se31 = np.zeros((128, 128), np.float32)
    Se31[0:2] = T[N:XR, cols31]

    # odd tiles use the packing rolled by 64 partitions so segment
    # gather-DMAs can pair port-disjoint even/odd partition ranges
    def roll(S):
        return np.roll(S, 64, axis=1)

    w = np.concatenate([Sm, Se, roll(Sm), roll(Se), roll(Sm31), roll(Se31),
                        U], axis=1).astype(np.float16)
    _stat_cache[key] = w
    return w


def build_program(loop_iters=None, variant="full"):
    """Build + compile the per-core Bass program (identical on all cores)."""
    key = (loop_iters, variant)
    if key in _nc_cache:
        return _nc_cache[key]
    mm_only = variant == "mm"

    nc = bacc.Bacc("TRN2", target_bir_lowering=False, debug=False)
    x_d = nc.dram_tensor("x", [XR, RPC], F16, kind="ExternalInput").ap()
    w_d = nc.dram_tensor("w", [P, 7 * P], F16, kind="ExternalInput").ap()
    y_d = nc.dram_tensor("y", [N, RPC], F16, kind="ExternalOutput").ap()
    a5s_d = nc.dram_tensor("a5s", [P, RPC], F16, kind="Internal").ap()

    with tile.TileContext(nc) as tc:
        with tc.tile_pool(name="sb", bufs=1) as sb, \
             tc.tile_pool(name="ps", bufs=8, space="PSUM") as ps:
            x_t = sb.tile([P, 33 * RPC], F16, name="x_t")
            st_t = sb.tile([P, NT * RPC], F16, name="st_t")
            w_t = sb.tile([P, 7 * P], F16, name="w_t")
            a5_t = sb.tile([P, RPC], F16, name="a5_t")
            p2_t = sb.tile([P, RPC], F16, name="p2_t")

            def xt(J):
                return x_t[:, J * RPC:(J + 1) * RPC]

            def stt(J):
                return st_t[:, J * RPC:(J + 1) * RPC]

            def body(_iv=None):
                nc.sync.dma_start(w_t[:], w_d)
                # batched input DMAs (HWDGE fixed cost is ~630 ns per
                # dma_start — per-tile DMAs serialize on it); small first
                # batch so the pipeline starts early
                batches = [(0, 2), (2, 6), (8, 8), (16, 8), (24, 8)]
                for j0, nj in batches:
                    sv = x_d[j0 * P:(j0 + nj) * P, :].rearrange(
                        "(j p) f -> p j f", p=P)
                    dv = x_t[:, j0 * RPC:(j0 + nj) * RPC].rearrange(
                        "p (j f) -> p j f", j=nj)
                    nc.sync.dma_start(dv, sv)
                # wrap tile 32: rows 0,1 = x cols 0,1; rest zero (only the
                # J=31 edge matmul reads it — keep off the startup path)
                nc.vector.memset(x_t[:, 32 * RPC:33 * RPC], 0.0)
                nc.sync.dma_start(x_t[0:2, 32 * RPC:33 * RPC], x_d[N:XR, :])

                # PE clock warmup: bridge PE busy-ness until batch 0 lands
                # (~2.5 us); the real matmul stream then sustains the HAM
                # streak to full clock
                pw = ps.tile([P, RPC], F32, name="pch", tag="ps")
                for _ in range(5):
                    nc.tensor.matmul(pw[:], w_t[:, 0:P], w_t[:, 0:512],
                                     start=True, stop=True)

                def a5_scatter(q):
                    """a5s row 4J+p <- staged tile J partition 64*(J%2)+p,
                    for tiles of parity q.  Tile auto-sems the DRAM RAW dep.
                    Sync ring: the scalar ring is congested with the late
                    gather chunks, which would delay the pass-2 readback."""
                    sv = st_t[64 * q:64 * q + 4, :].rearrange(
                        "p (j o f) -> p j o f", j=NT // 2, o=2)[:, :, q:q + 1, :]
                    dv = a5s_d.rearrange("(j o p) f -> p j o f",
                                         o=2, p=4)[:, :, q:q + 1, :]
                    nc.sync.dma_start(dv, sv)

                def seg_gather(pbase, seglen, j0, ntiles, dbase, eng=None):
                    """Drain one segment for tiles [j0, j0+ntiles) as an
                    even-tile + odd-tile DMA pair on disjoint SBUF ports."""
                    j2 = ntiles // 2
                    for par in (0, 1):
                        pb = (pbase + 64 * par) % P
                        sv = st_t[pb:pb + seglen,
                                  j0 * RPC:(j0 + ntiles) * RPC].rearrange(
                            "p (j q f) -> p j q f", j=j2, q=2)[:, :, par:par + 1, :]
                        dv = y_d[dbase + seglen * j0:
                                 dbase + seglen * (j0 + ntiles), :].rearrange(
                            "(j q p) f -> p j q f", q=2, p=seglen)[:, :, par:par + 1, :]
                        (eng or nc.scalar).dma_start(dv, sv)

                # pass 1
                for J in range(NT):
                    par = J % 2
                    mo = 2 * par if J < NT - 1 else 4
                    pt = ps.tile([P, RPC], F32, name="pch", tag="ps")
                    nc.tensor.matmul(pt[:], w_t[:, mo * P:(mo + 1) * P],
                                     xt(J), start=True, stop=False)
                    nc.tensor.matmul(pt[:], w_t[:, (mo + 1) * P:(mo + 2) * P],
                                     xt(J + 1), start=False, stop=True)
                    if mm_only:
                        continue
                    if par == 0:
                        nc.vector.tensor_copy(stt(J), pt[:])
                    else:
                        nc.scalar.copy(stt(J), pt[:])

                    # stream detail coeffs out as their staged tiles complete
                    # chunk boundaries skewed so only a small slice of the
                    # output remains after the last drain (short tail)
                    if J == 11:
                        seg_gather(64, 64, 0, 12, 2048)          # D1 c0
                    elif J == 19:
                        seg_gather(64, 64, 12, 8, 2048)          # D1 c1
                    elif J == 27:
                        seg_gather(64, 64, 20, 8, 2048)          # D1 c2
                    elif J == 31:
                        seg_gather(64, 64, 28, 4, 2048)          # D1 c3
                    if J == 19:
                        seg_gather(32, 32, 0, 20, 1024)          # D2 c0
                        # small segments ride SWDGE (Pool) — off the
                        # serialized HWDGE path (~630 ns per dma_start)
                        seg_gather(16, 16, 0, 20, 512, nc.gpsimd)    # D3
                        seg_gather(8, 8, 0, 20, 256, nc.gpsimd)      # D4
                        seg_gather(4, 4, 0, 20, 128, nc.gpsimd)      # D5
                    elif J == 31:
                        seg_gather(32, 32, 20, 12, 1024)         # D2 c1
                        seg_gather(16, 16, 20, 12, 512, nc.gpsimd)   # D3
                        seg_gather(8, 8, 20, 12, 256, nc.gpsimd)     # D4
                        seg_gather(4, 4, 20, 12, 128, nc.gpsimd)     # D5
                    if J == 30 and not mm_only:
                        a5_scatter(0)

                # (a5 odd-tile scatter + readback; even scatter went at J=30)
                if mm_only:
                    return
                a5_scatter(1)
                nc.sync.dma_start(a5_t[:], a5s_d)

                # pass 2: levels 5-10 in one dense matmul
                p2 = ps.tile([P, RPC], F32, name="pch", tag="ps")
                nc.tensor.matmul(p2[:], w_t[:, 6 * P:7 * P], a5_t[:],
                                 start=True, stop=True)
                nc.vector.tensor_copy(p2_t[:], p2[:])
                nc.scalar.dma_start(y_d[0:P, :], p2_t[:])

            if loop_iters is None:
                body()
            else:
                with tc.For_i(0, loop_iters, 1,
                              hint_engines=(mybir.EngineType.PE,)) as iv:
                    body(iv)

    nc.compile()
    _nc_cache[key] = nc
    return nc


def make_in_maps(x, W=None):
    """Host prep: per-core transposed fp16 inputs + stationaries."""
    x = np.asarray(x, np.float32)
    w_np = _stationaries(W)
    in_maps = []
    for c in range(NCORES):
        xc = x[c * RPC:(c + 1) * RPC]
        xT = np.empty((XR, RPC), np.float16)
        xT[:N] = np.ascontiguousarray(xc.T, dtype=np.float16)
        xT[N] = xc[:, 0].astype(np.float16)
        xT[N + 1] = xc[:, 1].astype(np.float16)
        in_maps.append({"x": xT, "w": w_np})
    return in_maps


def kernel(input, W=None, **_unused):
    x = np.asarray(input, np.float32)
    assert x.shape == (B, N), x.shape
    in_maps = make_in_maps(x, W)
    nc = build_program()
    res = run_bass_kernel_spmd(nc, in_maps, core_ids=list(range(NCORES)))
    out = np.empty((B, N), np.float32)
    for c in range(NCORES):
        out[c * RPC:(c + 1) * RPC] = res.results[c]["y"].T.astype(np.float32)
    return out
